# revision 4
# baseline (speedup 1.0000x reference)
import numpy as np

N = 1024
C = 256
P = 128
NCORES = 8
IB = N // NCORES
SLOPE = 0.2
MASK_BIG = 1.0e30
ACT_MOD, ACT_CNT = 7, 2

_CACHE = {}


def _split_excess_waits(nc, max_waits=1):
    from concourse import mybir

    cnt = 0
    for f in nc.m.functions:
        for b in f.blocks:
            insts = b.instructions
            i = 0
            while i < len(insts):
                inst = insts[i]
                si = getattr(inst, "sync_info", None)
                if si is not None and si.on_wait and len(si.on_wait) > max_waits:
                    waits = list(si.on_wait)
                    extra, keep = waits[:-max_waits], waits[-max_waits:]
                    new_nops = []
                    for k in range(0, len(extra), max_waits):
                        cnt += 1
                        nop = mybir.InstNoOp(
                            name=f"I-waitsplit-{cnt}", ins=[], outs=[]
                        )
                        nop.engine = inst.engine
                        nop.sync_info = mybir.SyncInfo(
                            on_wait=extra[k : k + max_waits], on_update=[]
                        )
                        new_nops.append(nop)
                    inst.sync_info = mybir.SyncInfo(
                        on_wait=keep, on_update=list(si.on_update)
                    )
                    for j, nop in enumerate(new_nops):
                        insts.insert(i + j, nop)
                    i += len(new_nops)
                i += 1
    return cnt


def _build_nc():
    import concourse.bass as bass
    import concourse.tile as tile
    from concourse import mybir

    f32 = mybir.dt.float32
    f16 = mybir.dt.float16
    bf16 = mybir.dt.bfloat16
    i32 = mybir.dt.int32
    AF = mybir.ActivationFunctionType
    OP = mybir.AluOpType

    nc = bass.Bass(trn_type="TRN2", debug=False)

    d_nodesT = nc.dram_tensor("nodesT", [C, N], f32, kind="ExternalInput")
    d_nodesTmy = nc.dram_tensor("nodesT_my", [C, IB], f32, kind="ExternalInput")
    d_adj = nc.dram_tensor("adj_my", [IB, N], i32, kind="ExternalInput")
    d_wsT = nc.dram_tensor("WsrcT", [C, C], f32, kind="ExternalInput")
    d_wtT = nc.dram_tensor("WtgtT", [C, C], f32, kind="ExternalInput")
    d_bs2 = nc.dram_tensor("b_src2", [P, 2], f32, kind="ExternalInput")
    d_bt2 = nc.dram_tensor("b_tgt2", [P, 2], f32, kind="ExternalInput")
    d_a2 = nc.dram_tensor("a_2", [P, 2], f32, kind="ExternalInput")
    d_btrow = nc.dram_tensor("b_tgt_row", [1, C], f32, kind="ExternalInput")
    d_acols = nc.dram_tensor("a_cols", [2, P, 2 * P], bf16, kind="ExternalInput")
    d_idf = nc.dram_tensor("id_f32", [P, P], f32, kind="ExternalInput")
    d_idb = nc.dram_tensor("id_bf16", [P, P], bf16, kind="ExternalInput")
    d_out = nc.dram_tensor("out_my", [IB, C], f32, kind="ExternalOutput")

    with tile.TileContext(nc) as tc:
        with (
            tc.tile_pool(name="singles", bufs=1) as singles,
            tc.tile_pool(name="zpool", bufs=3) as zpool,
            tc.tile_pool(name="psS", bufs=1, space="PSUM") as psS,
            tc.tile_pool(name="psT", bufs=2, space="PSUM") as psT,
        ):
            vT0 = singles.tile([P, N], f32)
            vT1 = singles.tile([P, N], f32)
            nc.sync.dma_start(out=vT0, in_=d_nodesT.ap()[0:P, :])
            nc.sync.dma_start(out=vT1, in_=d_nodesT.ap()[P : 2 * P, :])
            vT = [vT0, vT1]

            uTin0 = singles.tile([P, IB], f32)
            uTin1 = singles.tile([P, IB], f32)
            nc.sync.dma_start(out=uTin0, in_=d_nodesTmy.ap()[0:P, :])
            nc.sync.dma_start(out=uTin1, in_=d_nodesTmy.ap()[P : 2 * P, :])
            uTin = [uTin0, uTin1]

            wsT0 = singles.tile([P, C], f32)
            wsT1 = singles.tile([P, C], f32)
            nc.sync.dma_start(out=wsT0, in_=d_wsT.ap()[0:P, :])
            nc.sync.dma_start(out=wsT1, in_=d_wsT.ap()[P : 2 * P, :])
            wsT = [wsT0, wsT1]

            wtT0 = singles.tile([P, C], f32)
            wtT1 = singles.tile([P, C], f32)
            nc.sync.dma_start(out=wtT0, in_=d_wtT.ap()[0:P, :])
            nc.sync.dma_start(out=wtT1, in_=d_wtT.ap()[P : 2 * P, :])
            wtT = [wtT0, wtT1]

            adj_sb = singles.tile([IB, N], i32)
            nc.sync.dma_start(out=adj_sb, in_=d_adj.ap())

            bs2 = singles.tile([P, 2], f32)
            bt2 = singles.tile([P, 2], f32)
            a2 = singles.tile([P, 2], f32)
            nc.sync.dma_start(out=bs2, in_=d_bs2.ap())
            nc.sync.dma_start(out=bt2, in_=d_bt2.ap())
            nc.sync.dma_start(out=a2, in_=d_a2.ap())

            acol0 = singles.tile([P, 2 * P], bf16)
            acol1 = singles.tile([P, 2 * P], bf16)
            nc.sync.dma_start(out=acol0, in_=d_acols.ap()[0, :, :])
            nc.sync.dma_start(out=acol1, in_=d_acols.ap()[1, :, :])
            acol = [acol0, acol1]

            idf = singles.tile([P, P], f32)
            idb = singles.tile([P, P], bf16)
            nc.sync.dma_start(out=idf, in_=d_idf.ap())
            nc.sync.dma_start(out=idb, in_=d_idb.ap())

            bb = singles.tile([P, C], f32)
            nc.sync.dma_start(out=bb, in_=d_btrow.ap().to_broadcast([P, C]))

            m_bf = singles.tile([IB, N], bf16)
            nc.vector.tensor_scalar(
                out=m_bf, in0=adj_sb, scalar1=1.0, scalar2=MASK_BIG,
                op0=OP.subtract, op1=OP.mult,
            )

            gtT0 = singles.tile([P, N], f32)
            gtT1 = singles.tile([P, N], f32)
            gtT = [gtT0, gtT1]
            v16_0 = singles.tile([P, N], f16)
            v16_1 = singles.tile([P, N], f16)
            v16 = [v16_0, v16_1]
            for cb in range(2):
                for jt in range(2):
                    ps = psT.tile([P, 512], f32, tag="ps")
                    for kd in range(2):
                        nc.tensor.matmul(
                            ps,
                            lhsT=wtT[kd][:, cb * P : (cb + 1) * P],
                            rhs=vT[kd][:, jt * 512 : (jt + 1) * 512],
                            start=(kd == 0),
                            stop=(kd == 1),
                        )
                    nc.vector.tensor_scalar(
                        out=gtT[cb][:, jt * 512 : (jt + 1) * 512],
                        in0=ps, scalar1=bt2[:, cb : cb + 1], scalar2=None,
                        op0=OP.add,
                    )
                    nc.scalar.activation(
                        out=v16[cb][:, jt * 512 : (jt + 1) * 512],
                        in_=ps, func=AF.Identity,
                        bias=bt2[:, cb : cb + 1], scale=1.0,
                    )

            u_f32 = singles.tile([P, 2 * IB], f32)
            for cb in range(2):
                ps = psT.tile([P, IB], f32, tag="ps")
                for kd in range(2):
                    nc.tensor.matmul(
                        ps,
                        lhsT=wsT[kd][:, cb * P : (cb + 1) * P],
                        rhs=uTin[kd],
                        start=(kd == 0),
                        stop=(kd == 1),
                    )
                nc.vector.tensor_scalar(
                    out=u_f32[:, cb * IB : (cb + 1) * IB],
                    in0=ps, scalar1=bs2[:, cb : cb + 1], scalar2=None,
                    op0=OP.add,
                )

            psu = psT.tile([1, IB], f32, tag="ps")
            for cb in range(2):
                nc.tensor.matmul(
                    psu,
                    lhsT=a2[:, cb : cb + 1],
                    rhs=u_f32[:, cb * IB : (cb + 1) * IB],
                    start=(cb == 0),
                    stop=(cb == 1),
                )
            su_row = singles.tile([1, IB], f32)
            nc.scalar.mul(out=su_row, in_=psu, mul=SLOPE)

            sv_row = singles.tile([1, N], f32)
            for jt in range(2):
                psv = psT.tile([1, 512], f32, tag="ps")
                for cb in range(2):
                    nc.tensor.matmul(
                        psv,
                        lhsT=a2[:, cb : cb + 1],
                        rhs=gtT[cb][:, jt * 512 : (jt + 1) * 512],
                        start=(cb == 0),
                        stop=(cb == 1),
                    )
                nc.scalar.mul(
                    out=sv_row[:, jt * 512 : (jt + 1) * 512], in_=psv, mul=SLOPE
                )

            gU = singles.tile([P, 8 * C], f32)
            for jb in range(8):
                ps = psT.tile([P, C], f32, tag="ps")
                for kd in range(2):
                    nc.tensor.matmul(
                        ps,
                        lhsT=vT[kd][:, jb * P : (jb + 1) * P],
                        rhs=wtT[kd],
                        start=(kd == 0),
                        stop=(kd == 1),
                    )
                if jb % 2 == 0:
                    nc.scalar.copy(out=gU[:, jb * C : (jb + 1) * C], in_=ps)
                else:
                    nc.vector.tensor_copy(out=gU[:, jb * C : (jb + 1) * C], in_=ps)

            ones_row = singles.tile([1, P], f32)
            nc.vector.memset(ones_row, 1.0)
            ones512 = singles.tile([1, 512], f32)
            nc.vector.memset(ones512, 1.0)

            S = psS.tile([P, N], f32)

            for jt in range(2):
                nc.tensor.matmul(
                    S[:, jt * 512 : (jt + 1) * 512],
                    lhsT=idb,
                    rhs=m_bf[:, jt * 512 : (jt + 1) * 512],
                    start=True,
                    stop=False,
                    skip_group_check=True,
                )
            for jt in range(2):
                nc.tensor.matmul(
                    S[:, jt * 512 : (jt + 1) * 512],
                    lhsT=su_row,
                    rhs=ones512,
                    start=False,
                    stop=False,
                    skip_group_check=True,
                )
                nc.tensor.matmul(
                    S[:, jt * 512 : (jt + 1) * 512],
                    lhsT=ones_row,
                    rhs=sv_row[:, jt * 512 : (jt + 1) * 512],
                    start=False,
                    stop=False,
                    skip_group_check=True,
                )

            for i in range(IB):
                on_act = (i % ACT_MOD) < ACT_CNT
                for cb in range(2):
                    z = zpool.tile([P, N], bf16, tag=f"z{cb}")
                    bias_ap = u_f32[:, cb * IB + i : cb * IB + i + 1]
                    if on_act:
                        nc.scalar.activation(
                            out=z, in_=v16[cb], func=AF.Relu,
                            bias=bias_ap, scale=1.0,
                        )
                    else:
                        nc.vector.tensor_scalar(
                            out=z, in0=v16[cb], scalar1=bias_ap, scalar2=0.0,
                            op0=OP.add, op1=OP.max,
                        )
                    last = (i == IB - 1) and (cb == 1)
                    for jt in range(2):
                        nc.tensor.matmul(
                            S[:, jt * 512 : (jt + 1) * 512],
                            lhsT=acol[cb][:, P - i : 2 * P - i],
                            rhs=z[:, jt * 512 : (jt + 1) * 512],
                            start=False,
                            stop=last and (jt == 1),
                            skip_group_check=True,
                        )

            E = singles.tile([P, N], f32)
            rowsum = singles.tile([P, 1], f32)
            nc.scalar.activation(
                out=E, in_=S, func=AF.Exp, bias=0.0, scale=1.0, accum_out=rowsum
            )
            rinv = singles.tile([P, 1], f32)
            nc.vector.reciprocal(out=rinv, in_=rowsum)

            ET = singles.tile([P, N], f32)
            for jb in range(8):
                pt = psT.tile([P, P], f32, tag="ps")
                nc.tensor.transpose(pt, E[:, jb * P : (jb + 1) * P], idf)
                if jb % 2 == 0:
                    nc.vector.tensor_copy(out=ET[:, jb * P : (jb + 1) * P], in_=pt)
                else:
                    nc.scalar.copy(out=ET[:, jb * P : (jb + 1) * P], in_=pt)

            po = psT.tile([P, C], f32, tag="ps")
            for jb in range(8):
                nc.tensor.matmul(
                    po,
                    lhsT=ET[:, jb * P : (jb + 1) * P],
                    rhs=gU[:, jb * C : (jb + 1) * C],
                    start=(jb == 0),
                    stop=(jb == 7),
                )
            out_sb = singles.tile([IB, C], f32)
            nc.vector.tensor_scalar(
                out=out_sb, in0=po, scalar1=rinv, scalar2=None, op0=OP.mult
            )
            nc.vector.tensor_add(out=out_sb, in0=out_sb, in1=bb)
            nc.sync.dma_start(out=d_out.ap(), in_=out_sb)

    return nc


def _get_nc():
    if "nc" not in _CACHE:
        _CACHE["nc"] = _build_nc()
    return _CACHE["nc"]


def make_in_maps(nodes, adj_mat, W_src_w, W_src_b, W_tgt_w, W_tgt_b, a_w):
    import ml_dtypes

    f32 = np.float32
    nodesT = np.ascontiguousarray(nodes.T, dtype=f32)
    WsrcT = np.ascontiguousarray(W_src_w.T, dtype=f32)
    WtgtT = np.ascontiguousarray(W_tgt_w.T, dtype=f32)
    bs2 = np.ascontiguousarray(np.asarray(W_src_b, f32).reshape(2, P).T)
    bt2 = np.ascontiguousarray(np.asarray(W_tgt_b, f32).reshape(2, P).T)
    a2 = np.ascontiguousarray(np.asarray(a_w, f32).reshape(2, P).T)
    btrow = np.asarray(W_tgt_b, f32).reshape(1, C)
    acols = np.zeros((2, P, 2 * P), np.float32)
    for cb in range(2):
        acols[cb, :, P] = (1.0 - SLOPE) * np.asarray(a_w, f32)[cb * P : (cb + 1) * P]
    acols = acols.astype(ml_dtypes.bfloat16)
    idf = np.eye(P, dtype=f32)
    idb = np.eye(P, dtype=ml_dtypes.bfloat16)

    in_maps = []
    for k in range(NCORES):
        in_maps.append(
            {
                "nodesT": nodesT,
                "nodesT_my": np.ascontiguousarray(nodesT[:, k * IB : (k + 1) * IB]),
                "adj_my": np.ascontiguousarray(adj_mat[k * IB : (k + 1) * IB, :], np.int32),
                "WsrcT": WsrcT,
                "WtgtT": WtgtT,
                "b_src2": bs2,
                "b_tgt2": bt2,
                "a_2": a2,
                "b_tgt_row": btrow,
                "a_cols": acols,
                "id_f32": idf,
                "id_bf16": idb,
            }
        )
    return in_maps


def kernel(nodes, adj_mat, W_src_w, W_src_b, W_tgt_w, W_tgt_b, a_w, _trace=False):
    from concourse.bass_utils import run_bass_kernel_spmd

    nc = _get_nc()
    if not _CACHE.get("split_done"):
        _split_excess_waits(nc)
        _CACHE["split_done"] = True
    in_maps = make_in_maps(nodes, adj_mat, W_src_w, W_src_b, W_tgt_w, W_tgt_b, a_w)
    res = run_bass_kernel_spmd(nc, in_maps, core_ids=list(range(NCORES)), trace=_trace)
    out = np.concatenate([res.results[k]["out_my"] for k in range(NCORES)], axis=0)
    if _trace:
        _CACHE["last_results"] = res
    return out.astype(np.float32)
